# revision 15
# baseline (speedup 1.0000x reference)
"""Performer (FAVOR+) attention block on 8 Trainium2 NeuronCores.

Math (per batch b; the 1/sqrt(m) normalizations cancel between
numerator and denominator and a 64x scale is folded into the exp so
fp8 values stay in the normal range; eps is rescaled accordingly):
    kp' = 64*exp(k @ w.T - |k|^2/2)               [T, m]
    qp' = 64*exp(q @ w.T - |q|^2/2)               [T, m]
    ksum = kp'.sum(axis=0)/64                     [m]
    kptv'' = v.T @ kp'                            [d, m]
    C''  = kptv''.T @ proj_w.T                    [m, d]
    out  = (qp' @ C'') / (64*(qp' @ ksum) + 4096*m*eps)

Sharding: 8 cores = 4 batches x 2 token-halves; pairwise AllReduce of
C''+ksum (cores 2b, 2b+1); the q-side hides the collective; the tail
is just the out matmuls.

All matmul operands are fp8e4 with perf_mode=DoubleRow (K=256 per
matmul, ~1.7x bf16 throughput); accumulation is fp32 in PSUM. Inputs
are cast to fp8 and PAIR-INTERLEAVED on the host so every DoubleRow
operand is a contiguous [128, 2, N] access pattern (middle dim = the
two 128-row contraction planes). HBM I/O is fp8 in / bf16 out.

DMA: only 3 issue queues (sync/gpsimd/scalar), ~150 GB/s each sharing
~358 GB/s HBM; the critical set (k r0-3 + w) goes first across all 3.
"""

import math

import numpy as np
import ml_dtypes

import concourse.bass as bass
import concourse.mybir as mybir
import concourse.tile as tile
from concourse import bacc, bass_utils

F32 = mybir.dt.float32
BF16 = mybir.dt.bfloat16
FP8 = mybir.dt.float8e4
AF = mybir.ActivationFunctionType
DR = mybir.MatmulPerfMode.DoubleRow
BF16_NP = ml_dtypes.bfloat16
FP8_NP = ml_dtypes.float8_e4m3

N_CORES = 8
B, T, D_MODEL, M = 4, 4096, 1024, 512
TC = T // 2                       # tokens per core (keys AND queries)
DT = D_MODEL // 128               # 8 d tiles
MT = M // 128                     # 4 m tiles
RC = TC // 128                    # 16 token tiles per core
NCH = TC // 512                   # 4 512-token chunks per core
EPS_M = 1e-8 * M
LOG64 = math.log(64.0)            # folded into the exp bias
KV_SCALE = 1.0 / 128.0            # kv8 = kptv_u/2 (fp8 range)
KS_SCALE = 1.0 / 4096.0           # ksum8 = ksum_u/64 (fp8 range)
DIV_SCALE = 32.0                  # out = po / (32*pD + 32*m*eps)
DIV_BIAS = 32.0 * EPS_M
CC_GROUPS = [[0, 1], [2, 3], [4, 5], [6, 7]]
CC_COFF = 512
CC_COLS = CC_COFF + MT * D_MODEL
N_DUMMY = 8


def _pair(ap):
    """View a [128, 2*N] slice as the DoubleRow [128, 2, N] operand."""
    return ap.rearrange("p (o n) -> p o n", o=2)


def _build_program():
    nc = bacc.Bacc("TRN2", target_bir_lowering=False, debug=False,
                   num_devices=N_CORES)

    k_d = nc.dram_tensor("k8", [128, RC * 1024], FP8, kind="ExternalInput")
    q_d = nc.dram_tensor("q8", [128, DT * 2048], FP8, kind="ExternalInput")
    v_d = nc.dram_tensor("v8", [128, RC * 1024], FP8, kind="ExternalInput")
    wk_d = nc.dram_tensor("wk8", [128, DT * M], FP8, kind="ExternalInput")
    wq_d = nc.dram_tensor("wq8", [128, DT * M], FP8, kind="ExternalInput")
    pw_d = nc.dram_tensor("pw8", [128, DT * D_MODEL], FP8,
                          kind="ExternalInput")
    id_d = nc.dram_tensor("ident", [128, 128], BF16, kind="ExternalInput")
    out_d = nc.dram_tensor("out", [TC, D_MODEL], BF16, kind="ExternalOutput")

    with tile.TileContext(nc) as tc:
        with (
            tc.tile_pool(name="res", bufs=1) as res,
            tc.tile_pool(name="sqstream", bufs=2) as sqstream,
            tc.tile_pool(name="small", bufs=8) as small,
            tc.tile_pool(name="outp", bufs=3) as outp,
            tc.tile_pool(name="dram", bufs=1, space="DRAM") as dram,
        ):
            # ---- resident SBUF tensors (matmul operands fp8) ----
            # kt8[p, r*1024 + j*256 + o*128 + t'] = k[(2j+o)*128+p, r*128+t']
            kt8 = res.tile([128, RC * 1024], FP8, tag="kt8")
            # qt8[p, j*4096 + c*1024 + o*512 + t'] = qT[(2j+o)*128+p, c*512+t']
            qt8 = res.tile([128, DT * 2048], FP8, tag="qt8")
            # vt8[p, rr*2048 + dt*256 + o*128 + d'] = v[(2rr+o)*128+p, dt*128+d']
            vt8 = res.tile([128, RC * 1024], FP8, tag="vt8")
            # wk8[p, j*1024 + o*512 + m] = wT[(2j+o)*128+p, m]
            wk8 = res.tile([128, DT * M], FP8, tag="wk8")
            # wq8[p, j*1024 + mt*256 + o*128 + m'] = wT[(2j+o)*128+p, mt*128+m']
            wq8 = res.tile([128, DT * M], FP8, tag="wq8")
            # pw8[p, j*2048 + h*1024 + o*512 + n'] = pwT[(2j+o)*128+p, h*512+n']
            pw8 = res.tile([128, DT * D_MODEL], FP8, tag="pw8")
            # kp8[p, r*512 + m] = kp'[r*128+p, m]
            kp8 = res.tile([128, RC * M], FP8, tag="kp8")
            # qp8[p, j*4096 + r*256 + o*128 + t'] = qp'T[(2j+o)*128+p, r*128+t']
            qp8 = res.tile([128, MT * TC], FP8, tag="qp8")
            # kv8[p, j*1024 + mt*256 + o*128 + m'] = kptv''[(2j+o)*128+p, mt*128+m']
            kv8 = res.tile([128, DT * M], FP8, tag="kv8")
            # C8[p, j*2048 + h*1024 + o*512 + n'] = C''[(2j+o)*128+p, h*512+n']
            C8 = res.tile([128, MT * D_MODEL], FP8, tag="C8")
            ksum8 = res.tile([128, MT], FP8, tag="ksum8")
            xdc_k = res.tile([128, RC], F32, tag="xdc_k")
            xdT_q = res.tile([1, TC], BF16, tag="xdT_q")
            ident = res.tile([128, 128], BF16, tag="ident")
            ones_pair8 = res.tile([128, 32], FP8, tag="ones_pair8")
            ones_row = res.tile([1, 128], BF16, tag="ones_row")
            neghalf_col = res.tile([128, 1], BF16, tag="neghalf_col")
            junkA = res.tile([128, 1024], BF16, tag="junkA")
            junkB = res.tile([128, 1024], BF16, tag="junkB")

            cc_in = dram.tile([128, CC_COLS], FP8, tag="cc_in")
            cc_out = dram.tile([128, CC_COLS], FP8, tag="cc_out")

            # ---- loads: critical set (k r0-3, w) across all 3 queues ----
            nc.gpsimd.memset(ones_pair8[:], 1.0)
            nc.gpsimd.memset(neghalf_col[:], -0.5)
            nc.gpsimd.memset(ones_row[:], 1.0)
            nc.scalar.dma_start(wk8[:], wk_d[:, :])
            nc.scalar.dma_start(ident[:], id_d[:, :])
            # warm the exp table early (off the critical path)
            wexp = small.tile([128, 1], BF16, tag="wexp")
            nc.scalar.activation(wexp[:], neghalf_col[:], AF.Exp)
            # k: 16 single-r tiles round-robin across the 3 queues, in the
            # order the wtx loop consumes them (~33 GB/s per queue)
            _kq = [nc.sync, nc.gpsimd, nc.scalar]
            for r in range(RC):
                _kq[r % 3].dma_start(kt8[:, r * 1024:(r + 1) * 1024],
                                     k_d[:, r * 1024:(r + 1) * 1024])
            for i in range(4):
                nc.scalar.dma_start(vt8[:, i * 4096:(i + 1) * 4096],
                                    v_d[:, i * 4096:(i + 1) * 4096])
            nc.gpsimd.dma_start(wq8[:], wq_d[:, :])
            for i in range(2):
                nc.sync.dma_start(qt8[:, i * 8192:(i + 1) * 8192],
                                  q_d[:, i * 8192:(i + 1) * 8192])
            for i in range(2):
                nc.gpsimd.dma_start(pw8[:, i * 4096:(i + 1) * 4096],
                                    pw_d[:, i * 4096:(i + 1) * 4096])
            nc.gpsimd.memset(junkA[:], 0.0)

            # ================= K side (token-major, DoubleRow) ==========
            with (
                tc.tile_pool(name="psum_wtxk", bufs=5,
                             space=bass.MemorySpace.PSUM) as psum_wtx,
                tc.tile_pool(name="psum_gram", bufs=2,
                             space=bass.MemorySpace.PSUM) as psum_gram,
                tc.tile_pool(name="psum_ks", bufs=1,
                             space=bass.MemorySpace.PSUM) as psum_ks,
            ):
                ks = psum_ks.tile([16, M], F32, tag="ks")
                for r in range(RC):
                    # -|k|^2/2 via the k-Gram diagonal, on the PE: the 4
                    # gram matmuls reuse the same stationary kt pairs as
                    # the wtx matmuls below; the diagonal is extracted by
                    # an identity mask (DVE) + activation accum_out with
                    # scale=-1/2 and bias=log(64)/128 (summed 128x).
                    gram = psum_gram.tile([128, 128], F32, tag="gram")
                    for j in range(4):
                        kpair = _pair(kt8[:, r * 1024 + j * 256:
                                          r * 1024 + (j + 1) * 256])
                        nc.tensor.matmul(gram[:], kpair, kpair,
                                         start=(j == 0), stop=(j == 3),
                                         perf_mode=DR)
                    dv = sqstream.tile([128, 128], BF16, tag="dv", bufs=3)
                    nc.vector.tensor_mul(dv[:], gram[:], ident[:])
                    scr = sqstream.tile([128, 128], BF16, tag="scr", bufs=2)
                    nc.scalar.activation(scr[:], dv[:], AF.Copy, scale=-0.5,
                                         bias=LOG64 / 128.0,
                                         accum_out=xdc_k[:, r:r + 1])
                    # wtx[t, m] over 4 dt-pairs, DoubleRow
                    ps = psum_wtx.tile([128, M], F32, tag="wtx")
                    for j in range(4):
                        nc.tensor.matmul(
                            ps[:],
                            _pair(kt8[:, r * 1024 + j * 256:
                                      r * 1024 + (j + 1) * 256]),
                            _pair(wk8[:, j * 1024:(j + 1) * 1024]),
                            start=(j == 0), stop=(j == 3), perf_mode=DR)
                    nc.scalar.activation(kp8[:, r * M:(r + 1) * M], ps[:],
                                         AF.Exp, bias=xdc_k[:, r:r + 1])
                    if r % 2 == 1:
                        rr = r // 2
                        nc.tensor.matmul(
                            ks[:], _pair(ones_pair8[:]),
                            _pair(kp8[:, rr * 1024:(rr + 1) * 1024]),
                            start=(rr == 0), stop=(rr == RC // 2 - 1),
                            perf_mode=DR)
                ks_st = small.tile([1, M], FP8, tag="ks_st")
                nc.scalar.activation(ks_st[:], ks[0:1, :], AF.Copy,
                                     scale=KS_SCALE)
                nc.sync.dma_start(cc_in[0:1, 0:M], ks_st[:])

            # ---- kptv'' d-major (v-stationary, DoubleRow): two waves
            # of 4 dt so wave0's drains overlap wave1's matmuls ----
            for wave in range(2):
                with tc.tile_pool(name=f"psum_kptv{wave}", bufs=1,
                                  space=bass.MemorySpace.PSUM) as psum_kptv:
                    pk = {dt: psum_kptv.tile([128, M], F32,
                                             tag=f"pk{dt}", name=f"pk{dt}")
                          for dt in range(4 * wave, 4 * wave + 4)}
                    for rr in range(RC // 2):
                        for dt in pk:
                            nc.tensor.matmul(
                                pk[dt][:],
                                _pair(vt8[:, rr * 2048 + dt * 256:
                                          rr * 2048 + (dt + 1) * 256]),
                                _pair(kp8[:, rr * 1024:(rr + 1) * 1024]),
                                start=(rr == 0), stop=(rr == RC // 2 - 1),
                                perf_mode=DR)
                    for dt in pk:
                        j, o = divmod(dt, 2)
                        for mt in range(MT):
                            nc.scalar.activation(
                                kv8[:, j * 1024 + mt * 256 + o * 128:
                                    j * 1024 + mt * 256 + (o + 1) * 128],
                                pk[dt][:, mt * 128:(mt + 1) * 128],
                                AF.Copy, scale=KV_SCALE)

            # ---- C'' partial = kptv''^T @ proj_w^T  [m, dout] ----
            with tc.tile_pool(name="psum_C", bufs=2,
                              space=bass.MemorySpace.PSUM) as psum_C:
                for mt in range(MT):
                    jq, oq = divmod(mt, 2)
                    pc = psum_C.tile([128, D_MODEL], F32, tag="pc")
                    for j in range(4):
                        lhs = _pair(kv8[:, j * 1024 + mt * 256:
                                        j * 1024 + (mt + 1) * 256])
                        for h in range(2):
                            nc.tensor.matmul(
                                pc[:, h * 512:(h + 1) * 512], lhs,
                                _pair(pw8[:, j * 2048 + h * 1024:
                                          j * 2048 + (h + 1) * 1024]),
                                start=(j == 0), stop=(j == 3), perf_mode=DR)
                    st = outp.tile([128, D_MODEL], FP8, tag="ccst",
                                   name="ccst", bufs=2)
                    nc.scalar.activation(st[:], pc[:], AF.Copy)
                    for h in range(2):
                        nc.sync.dma_start(
                            cc_in[:, CC_COFF + jq * 2048 + h * 1024 + oq * 512:
                                  CC_COFF + jq * 2048 + h * 1024 + (oq + 1) * 512],
                            st[:, h * 512:(h + 1) * 512])

            # ---- pairwise AllReduce of C'' + ksum (fp8 payload) ----
            nc.gpsimd.collective_compute(
                "AllReduce", mybir.AluOpType.add, replica_groups=CC_GROUPS,
                ins=[cc_in.opt()], outs=[cc_out.opt()])
            nc.sync.dma_start(
                ksum8[:],
                cc_out[0:1, 0:M].rearrange("a (mt p) -> p (mt a)", p=128))
            nc.sync.dma_start(C8[:], cc_out[:, CC_COFF:CC_COFF + MT * D_MODEL])

            # ================= Q side (hides the AllReduce) ============
            with (
                tc.tile_pool(name="psum_wtxq", bufs=4,
                             space=bass.MemorySpace.PSUM) as psum_wtx,
                tc.tile_pool(name="psum_xdq", bufs=2,
                             space=bass.MemorySpace.PSUM) as psum_xd,
            ):
                for c in range(NCH):
                    lvl = []
                    for j in range(4):
                        for o in range(2):
                            sq = sqstream.tile([128, 512], BF16, tag="qsq",
                                               name=f"qsq{j}{o}", bufs=8)
                            sl = qt8[:, j * 4096 + c * 1024 + o * 512:
                                     j * 4096 + c * 1024 + (o + 1) * 512]
                            nc.vector.tensor_mul(sq[:], sl, sl)
                            lvl.append(sq)
                    depth = 0
                    while len(lvl) > 1:
                        nxt = []
                        for i in range(0, len(lvl), 2):
                            s = sqstream.tile([128, 512], BF16,
                                              tag=f"qsa{depth}",
                                              name=f"qsa{depth}_{i}",
                                              bufs=max(2, 4 >> depth))
                            nc.vector.tensor_add(s[:], lvl[i][:], lvl[i + 1][:])
                            nxt.append(s)
                        lvl = nxt
                        depth += 1
                    xdp = psum_xd.tile([1, 512], F32, tag="xdq")
                    nc.tensor.matmul(xdp[:], neghalf_col[:], lvl[0][:],
                                     start=True, stop=True)
                    nc.scalar.activation(xdT_q[0:1, c * 512:(c + 1) * 512],
                                         xdp[:], AF.Copy, bias=LOG64)
                for mt in range(MT):
                    jq, oq = divmod(mt, 2)
                    for c in range(NCH):
                        wqp = psum_wtx.tile([128, 512], F32, tag="wq")
                        for j in range(4):
                            nc.tensor.matmul(
                                wqp[:],
                                _pair(wq8[:, j * 1024 + mt * 256:
                                          j * 1024 + (mt + 1) * 256]),
                                _pair(qt8[:, j * 4096 + c * 1024:
                                          j * 4096 + (c + 1) * 1024]),
                                start=(j == 0), stop=False, perf_mode=DR)
                        nc.tensor.matmul(wqp[:], ones_row[:],
                                         xdT_q[0:1, c * 512:(c + 1) * 512],
                                         start=False, stop=True)
                        for rl in range(4):
                            r = c * 4 + rl
                            nc.scalar.activation(
                                qp8[:, jq * 4096 + r * 256 + oq * 128:
                                    jq * 4096 + r * 256 + (oq + 1) * 128],
                                wqp[:, rl * 128:(rl + 1) * 128], AF.Exp)

            # ---- HAM warm-keeper: paced dummy matmuls (CC insurance) ----
            with tc.tile_pool(name="psum_dummy", bufs=2,
                              space=bass.MemorySpace.PSUM) as psum_dummy:
                for i in range(N_DUMMY):
                    src, dst = (junkA, junkB) if i % 2 == 0 else (junkB, junkA)
                    nc.vector.tensor_copy(dst[:], src[:])
                    dp = psum_dummy.tile([128, 16], F32, tag="dp")
                    nc.tensor.matmul(dp[:], ident[:, 0:128],
                                     dst[:, 0:16], start=True, stop=True)

            # ---- OUT: out = po / (64*pD + 4096*m*eps) ----
            with (
                tc.tile_pool(name="psum_o", bufs=3,
                             space=bass.MemorySpace.PSUM) as psum_o,
                tc.tile_pool(name="psum_D", bufs=2,
                             space=bass.MemorySpace.PSUM) as psum_D,
            ):
                for r in range(RC):
                    po = psum_o.tile([128, D_MODEL], F32, tag="po")
                    pD = psum_D.tile([128, 1], F32, tag="pD")
                    for j in range(2):
                        lhs = _pair(qp8[:, j * 4096 + r * 256:
                                        j * 4096 + (r + 1) * 256])
                        for h in range(2):
                            nc.tensor.matmul(
                                po[:, h * 512:(h + 1) * 512], lhs,
                                _pair(C8[:, j * 2048 + h * 1024:
                                         j * 2048 + (h + 1) * 1024]),
                                start=(j == 0), stop=(j == 1), perf_mode=DR)
                    for mt in range(MT):
                        jq, oq = divmod(mt, 2)
                        nc.tensor.matmul(
                            pD[:],
                            qp8[:, jq * 4096 + r * 256 + oq * 128:
                                jq * 4096 + r * 256 + (oq + 1) * 128],
                            ksum8[:, mt:mt + 1],
                            start=(mt == 0), stop=(mt == MT - 1))
                    Dp = small.tile([128, 1], F32, tag="Dp")
                    recD = small.tile([128, 1], F32, tag="recD")
                    nc.scalar.activation(Dp[:], pD[:], AF.Copy,
                                         scale=DIV_SCALE, bias=DIV_BIAS)
                    nc.vector.reciprocal(recD[:], Dp[:])
                    ot = outp.tile([128, D_MODEL], BF16, tag="ot")
                    for h in range(2):
                        nc.vector.tensor_scalar_mul(
                            ot[:, h * 512:(h + 1) * 512],
                            po[:, h * 512:(h + 1) * 512], recD[:])
                    nc.sync.dma_start(out_d[r * 128:(r + 1) * 128, :], ot[:])

    nc.compile()
    return nc


_NC_CACHE = None


def _get_program():
    global _NC_CACHE
    if _NC_CACHE is None:
        _NC_CACHE = _build_program()
    return _NC_CACHE


def _c(a):
    return np.ascontiguousarray(a)


def _make_in_maps(q, k, v, w, proj_w):
    wT = w.T.astype(FP8_NP)          # [1024, 512]
    pwT = proj_w.T.astype(FP8_NP)    # [1024, 1024]
    wk = _c(wT.reshape(4, 2, 128, 512).transpose(2, 0, 1, 3)
            .reshape(128, 4096))
    wq = _c(wT.reshape(4, 2, 128, 4, 128).transpose(2, 0, 3, 1, 4)
            .reshape(128, 4096))
    pw = _c(pwT.reshape(4, 2, 128, 2, 512).transpose(2, 0, 3, 1, 4)
            .reshape(128, 8192))
    in_maps = []
    for c in range(N_CORES):
        b, h = divmod(c, 2)
        sl = slice(h * TC, (h + 1) * TC)
        kT = k[b, sl].T.astype(FP8_NP)   # [1024, 2048]
        qT = q[b, sl].T.astype(FP8_NP)
        vv = v[b, sl].astype(FP8_NP)     # [2048, 1024]
        in_maps.append({
            "k8": _c(kT.reshape(4, 2, 128, 16, 128).transpose(2, 3, 0, 1, 4)
                     .reshape(128, 16384)),
            "q8": _c(qT.reshape(4, 2, 128, 4, 512).transpose(2, 0, 3, 1, 4)
                     .reshape(128, 16384)),
            "v8": _c(vv.reshape(8, 2, 128, 8, 128).transpose(2, 0, 3, 1, 4)
                     .reshape(128, 16384)),
            "wk8": wk,
            "ident": np.eye(128, dtype=BF16_NP),
            "wq8": wq,
            "pw8": pw,
        })
    return in_maps


def run(q, k, v, w, proj_w, trace=False, tmpdir=None):
    nc = _get_program()
    in_maps = _make_in_maps(q, k, v, w, proj_w)
    res = bass_utils.run_bass_kernel_spmd(
        nc, in_maps, core_ids=list(range(N_CORES)), trace=trace,
        tmpdir=tmpdir)
    out = np.empty((B, T, D_MODEL), dtype=np.float32)
    for c in range(N_CORES):
        b, h = divmod(c, 2)
        out[b, h * TC:(h + 1) * TC] = res.results[c]["out"].astype(np.float32)
    return out, res


def kernel(q, k, v, w, proj_w):
    out, _ = run(np.asarray(q, dtype=np.float32),
                 np.asarray(k, dtype=np.float32),
                 np.asarray(v, dtype=np.float32),
                 np.asarray(w, dtype=np.float32),
                 np.asarray(proj_w, dtype=np.float32))
    return out


# revision 16
# speedup vs baseline: 1.0800x; 1.0800x over previous
"""Performer (FAVOR+) attention block on 8 Trainium2 NeuronCores.

Math (per batch b; the 1/sqrt(m) normalizations cancel between
numerator and denominator and a 64x scale is folded into the exp so
fp8 values stay in the normal range; eps is rescaled accordingly):
    kp' = 64*exp(k @ w.T - |k|^2/2)               [T, m]
    qp' = 64*exp(q @ w.T - |q|^2/2)               [T, m]
    ksum = kp'.sum(axis=0)/64                     [m]
    kptv'' = v.T @ kp'                            [d, m]
    C''  = kptv''.T @ proj_w.T                    [m, d]
    out  = (qp' @ C'') / (64*(qp' @ ksum) + 4096*m*eps)

Sharding: 8 cores = 4 batches x 2 token-halves; pairwise AllReduce of
C''+ksum (cores 2b, 2b+1); the q-side hides the collective; the tail
is just the out matmuls.

All matmul operands are fp8e4 with perf_mode=DoubleRow (K=256 per
matmul, ~1.7x bf16 throughput); accumulation is fp32 in PSUM. Inputs
are cast to fp8 and PAIR-INTERLEAVED on the host so every DoubleRow
operand is a contiguous [128, 2, N] access pattern (middle dim = the
two 128-row contraction planes). HBM I/O is fp8 in / bf16 out.

DMA: only 3 issue queues (sync/gpsimd/scalar), ~150 GB/s each sharing
~358 GB/s HBM; the critical set (k r0-3 + w) goes first across all 3.
"""

import math

import numpy as np
import ml_dtypes

import concourse.bass as bass
import concourse.mybir as mybir
import concourse.tile as tile
from concourse import bacc, bass_utils

F32 = mybir.dt.float32
BF16 = mybir.dt.bfloat16
FP8 = mybir.dt.float8e4
AF = mybir.ActivationFunctionType
DR = mybir.MatmulPerfMode.DoubleRow
BF16_NP = ml_dtypes.bfloat16
FP8_NP = ml_dtypes.float8_e4m3

N_CORES = 8
B, T, D_MODEL, M = 4, 4096, 1024, 512
TC = T // 2                       # tokens per core (keys AND queries)
DT = D_MODEL // 128               # 8 d tiles
MT = M // 128                     # 4 m tiles
RC = TC // 128                    # 16 token tiles per core
NCH = TC // 512                   # 4 512-token chunks per core
EPS_M = 1e-8 * M
LOG64 = math.log(64.0)            # folded into the exp bias
KV_SCALE = 1.0 / 128.0            # kv8 = kptv_u/2 (fp8 range)
KS_SCALE = 1.0 / 4096.0           # ksum8 = ksum_u/64 (fp8 range)
DIV_SCALE = 32.0                  # out = po / (32*pD + 32*m*eps)
DIV_BIAS = 32.0 * EPS_M
CC_GROUPS = [[0, 1], [2, 3], [4, 5], [6, 7]]
CC_COFF = 512
CC_COLS = CC_COFF + MT * D_MODEL
N_DUMMY = 8


def _pair(ap):
    """View a [128, 2*N] slice as the DoubleRow [128, 2, N] operand."""
    return ap.rearrange("p (o n) -> p o n", o=2)


def _build_program():
    nc = bacc.Bacc("TRN2", target_bir_lowering=False, debug=False,
                   num_devices=N_CORES)

    k_d = nc.dram_tensor("k8", [128, RC * 1024], FP8, kind="ExternalInput")
    q_d = nc.dram_tensor("q8", [128, DT * 2048], FP8, kind="ExternalInput")
    v_d = nc.dram_tensor("v8", [128, RC * 1024], FP8, kind="ExternalInput")
    wk_d = nc.dram_tensor("wk8", [128, DT * M], FP8, kind="ExternalInput")
    wq_d = nc.dram_tensor("wq8", [128, DT * M], FP8, kind="ExternalInput")
    pw_d = nc.dram_tensor("pw8", [128, DT * D_MODEL], FP8,
                          kind="ExternalInput")
    id_d = nc.dram_tensor("ident", [128, 128], BF16, kind="ExternalInput")
    out_d = nc.dram_tensor("out", [TC, D_MODEL], BF16, kind="ExternalOutput")

    with tile.TileContext(nc) as tc:
        with (
            tc.tile_pool(name="res", bufs=1) as res,
            tc.tile_pool(name="sqstream", bufs=2) as sqstream,
            tc.tile_pool(name="small", bufs=8) as small,
            tc.tile_pool(name="outp", bufs=3) as outp,
            tc.tile_pool(name="dram", bufs=1, space="DRAM") as dram,
        ):
            # ---- resident SBUF tensors (matmul operands fp8) ----
            # kt8[p, r*1024 + j*256 + o*128 + t'] = k[(2j+o)*128+p, r*128+t']
            kt8 = res.tile([128, RC * 1024], FP8, tag="kt8")
            # qt8[p, j*4096 + c*1024 + o*512 + t'] = qT[(2j+o)*128+p, c*512+t']
            qt8 = res.tile([128, DT * 2048], FP8, tag="qt8")
            # vt8[p, rr*2048 + dt*256 + o*128 + d'] = v[(2rr+o)*128+p, dt*128+d']
            vt8 = res.tile([128, RC * 1024], FP8, tag="vt8")
            # wk8[p, j*1024 + o*512 + m] = wT[(2j+o)*128+p, m]
            wk8 = res.tile([128, DT * M], FP8, tag="wk8")
            # wq8[p, j*1024 + mt*256 + o*128 + m'] = wT[(2j+o)*128+p, mt*128+m']
            wq8 = res.tile([128, DT * M], FP8, tag="wq8")
            # pw8[p, j*2048 + h*1024 + o*512 + n'] = pwT[(2j+o)*128+p, h*512+n']
            pw8 = res.tile([128, DT * D_MODEL], FP8, tag="pw8")
            # kp8[p, r*512 + m] = kp'[r*128+p, m]
            kp8 = res.tile([128, RC * M], FP8, tag="kp8")
            # qp8[p, j*4096 + r*256 + o*128 + t'] = qp'T[(2j+o)*128+p, r*128+t']
            qp8 = res.tile([128, MT * TC], FP8, tag="qp8")
            # kv8[p, j*1024 + mt*256 + o*128 + m'] = kptv''[(2j+o)*128+p, mt*128+m']
            kv8 = res.tile([128, DT * M], FP8, tag="kv8")
            # C8[p, j*2048 + h*1024 + o*512 + n'] = C''[(2j+o)*128+p, h*512+n']
            C8 = res.tile([128, MT * D_MODEL], FP8, tag="C8")
            ksum8 = res.tile([128, MT], FP8, tag="ksum8")
            xdc_k = res.tile([128, RC], F32, tag="xdc_k")
            xdT_q = res.tile([1, TC], BF16, tag="xdT_q")
            ident = res.tile([128, 128], BF16, tag="ident")
            ones_pair8 = res.tile([128, 32], FP8, tag="ones_pair8")
            ones_row = res.tile([1, 128], BF16, tag="ones_row")
            neghalf_col = res.tile([128, 1], BF16, tag="neghalf_col")
            junkA = res.tile([128, 1024], BF16, tag="junkA")
            junkB = res.tile([128, 1024], BF16, tag="junkB")

            cc_in = dram.tile([128, CC_COLS], FP8, tag="cc_in")
            cc_out = dram.tile([128, CC_COLS], FP8, tag="cc_out")
            bar_in = dram.tile([1, 32], FP8, tag="bar_in")
            bar_out = dram.tile([1, 32], FP8, tag="bar_out")

            # ---- loads: critical set (k r0-3, w) across all 3 queues ----
            nc.gpsimd.memset(ones_pair8[:], 1.0)
            nc.gpsimd.memset(neghalf_col[:], -0.5)
            nc.gpsimd.memset(ones_row[:], 1.0)
            nc.scalar.dma_start(wk8[:], wk_d[:, :])
            nc.scalar.dma_start(ident[:], id_d[:, :])
            # warm the exp table early (off the critical path)
            wexp = small.tile([128, 1], BF16, tag="wexp")
            nc.scalar.activation(wexp[:], neghalf_col[:], AF.Exp)
            # k: 16 single-r tiles round-robin across the 3 queues, in the
            # order the wtx loop consumes them (~33 GB/s per queue)
            _kq = [nc.sync, nc.gpsimd, nc.scalar]
            for r in range(RC):
                _kq[r % 3].dma_start(kt8[:, r * 1024:(r + 1) * 1024],
                                     k_d[:, r * 1024:(r + 1) * 1024])
            for i in range(4):
                nc.scalar.dma_start(vt8[:, i * 4096:(i + 1) * 4096],
                                    v_d[:, i * 4096:(i + 1) * 4096])
            nc.gpsimd.dma_start(wq8[:], wq_d[:, :])
            for i in range(2):
                nc.sync.dma_start(qt8[:, i * 8192:(i + 1) * 8192],
                                  q_d[:, i * 8192:(i + 1) * 8192])
            for i in range(2):
                nc.gpsimd.dma_start(pw8[:, i * 4096:(i + 1) * 4096],
                                    pw_d[:, i * 4096:(i + 1) * 4096])
            nc.gpsimd.memset(junkA[:], 0.0)
            bar_sb = small.tile([1, 32], FP8, tag="bar_sb")
            nc.gpsimd.memset(bar_sb[:], 0.0)
            nc.sync.dma_start(bar_in[0:1, :], bar_sb[:])
            nc.gpsimd.collective_compute(
                "AllReduce", mybir.AluOpType.add, replica_groups=CC_GROUPS,
                ins=[bar_in.opt()], outs=[bar_out.opt()])

            # ================= K side (token-major, DoubleRow) ==========
            with (
                tc.tile_pool(name="psum_wtxk", bufs=5,
                             space=bass.MemorySpace.PSUM) as psum_wtx,
                tc.tile_pool(name="psum_gram", bufs=2,
                             space=bass.MemorySpace.PSUM) as psum_gram,
                tc.tile_pool(name="psum_ks", bufs=1,
                             space=bass.MemorySpace.PSUM) as psum_ks,
            ):
                ks = psum_ks.tile([16, M], F32, tag="ks")
                for r in range(RC):
                    # -|k|^2/2 via the k-Gram diagonal, on the PE: the 4
                    # gram matmuls reuse the same stationary kt pairs as
                    # the wtx matmuls below; the diagonal is extracted by
                    # an identity mask (DVE) + activation accum_out with
                    # scale=-1/2 and bias=log(64)/128 (summed 128x).
                    gram = psum_gram.tile([128, 128], F32, tag="gram")
                    for j in range(4):
                        kpair = _pair(kt8[:, r * 1024 + j * 256:
                                          r * 1024 + (j + 1) * 256])
                        nc.tensor.matmul(gram[:], kpair, kpair,
                                         start=(j == 0), stop=(j == 3),
                                         perf_mode=DR)
                    dv = sqstream.tile([128, 128], BF16, tag="dv", bufs=3)
                    nc.vector.tensor_mul(dv[:], gram[:], ident[:])
                    scr = sqstream.tile([128, 128], BF16, tag="scr", bufs=2)
                    nc.scalar.activation(scr[:], dv[:], AF.Copy, scale=-0.5,
                                         bias=LOG64 / 128.0,
                                         accum_out=xdc_k[:, r:r + 1])
                    # wtx[t, m] over 4 dt-pairs, DoubleRow
                    ps = psum_wtx.tile([128, M], F32, tag="wtx")
                    for j in range(4):
                        nc.tensor.matmul(
                            ps[:],
                            _pair(kt8[:, r * 1024 + j * 256:
                                      r * 1024 + (j + 1) * 256]),
                            _pair(wk8[:, j * 1024:(j + 1) * 1024]),
                            start=(j == 0), stop=(j == 3), perf_mode=DR)
                    nc.scalar.activation(kp8[:, r * M:(r + 1) * M], ps[:],
                                         AF.Exp, bias=xdc_k[:, r:r + 1])
                    if r % 2 == 1:
                        rr = r // 2
                        nc.tensor.matmul(
                            ks[:], _pair(ones_pair8[:]),
                            _pair(kp8[:, rr * 1024:(rr + 1) * 1024]),
                            start=(rr == 0), stop=(rr == RC // 2 - 1),
                            perf_mode=DR)
                ks_st = small.tile([1, M], FP8, tag="ks_st")
                nc.scalar.activation(ks_st[:], ks[0:1, :], AF.Copy,
                                     scale=KS_SCALE)
                nc.sync.dma_start(cc_in[0:1, 0:M], ks_st[:])

            # ---- kptv'' d-major (v-stationary, DoubleRow): two waves
            # of 4 dt so wave0's drains overlap wave1's matmuls ----
            for wave in range(2):
                with tc.tile_pool(name=f"psum_kptv{wave}", bufs=1,
                                  space=bass.MemorySpace.PSUM) as psum_kptv:
                    pk = {dt: psum_kptv.tile([128, M], F32,
                                             tag=f"pk{dt}", name=f"pk{dt}")
                          for dt in range(4 * wave, 4 * wave + 4)}
                    for rr in range(RC // 2):
                        for dt in pk:
                            nc.tensor.matmul(
                                pk[dt][:],
                                _pair(vt8[:, rr * 2048 + dt * 256:
                                          rr * 2048 + (dt + 1) * 256]),
                                _pair(kp8[:, rr * 1024:(rr + 1) * 1024]),
                                start=(rr == 0), stop=(rr == RC // 2 - 1),
                                perf_mode=DR)
                    for dt in pk:
                        j, o = divmod(dt, 2)
                        for mt in range(MT):
                            nc.scalar.activation(
                                kv8[:, j * 1024 + mt * 256 + o * 128:
                                    j * 1024 + mt * 256 + (o + 1) * 128],
                                pk[dt][:, mt * 128:(mt + 1) * 128],
                                AF.Copy, scale=KV_SCALE)

            # ---- C'' partial = kptv''^T @ proj_w^T  [m, dout] ----
            with tc.tile_pool(name="psum_C", bufs=2,
                              space=bass.MemorySpace.PSUM) as psum_C:
                for mt in range(MT):
                    jq, oq = divmod(mt, 2)
                    pc = psum_C.tile([128, D_MODEL], F32, tag="pc")
                    for j in range(4):
                        lhs = _pair(kv8[:, j * 1024 + mt * 256:
                                        j * 1024 + (mt + 1) * 256])
                        for h in range(2):
                            nc.tensor.matmul(
                                pc[:, h * 512:(h + 1) * 512], lhs,
                                _pair(pw8[:, j * 2048 + h * 1024:
                                          j * 2048 + (h + 1) * 1024]),
                                start=(j == 0), stop=(j == 3), perf_mode=DR)
                    st = outp.tile([128, D_MODEL], FP8, tag="ccst",
                                   name="ccst", bufs=2)
                    nc.scalar.activation(st[:], pc[:], AF.Copy)
                    for h in range(2):
                        nc.sync.dma_start(
                            cc_in[:, CC_COFF + jq * 2048 + h * 1024 + oq * 512:
                                  CC_COFF + jq * 2048 + h * 1024 + (oq + 1) * 512],
                            st[:, h * 512:(h + 1) * 512])

            # ---- pairwise AllReduce of C'' + ksum (fp8 payload) ----
            nc.gpsimd.collective_compute(
                "AllReduce", mybir.AluOpType.add, replica_groups=CC_GROUPS,
                ins=[cc_in.opt()], outs=[cc_out.opt()])
            nc.sync.dma_start(
                ksum8[:],
                cc_out[0:1, 0:M].rearrange("a (mt p) -> p (mt a)", p=128))
            nc.sync.dma_start(C8[:], cc_out[:, CC_COFF:CC_COFF + MT * D_MODEL])

            # ================= Q side (hides the AllReduce) ============
            with (
                tc.tile_pool(name="psum_wtxq", bufs=4,
                             space=bass.MemorySpace.PSUM) as psum_wtx,
                tc.tile_pool(name="psum_xdq", bufs=2,
                             space=bass.MemorySpace.PSUM) as psum_xd,
            ):
                for c in range(NCH):
                    lvl = []
                    for j in range(4):
                        for o in range(2):
                            sq = sqstream.tile([128, 512], BF16, tag="qsq",
                                               name=f"qsq{j}{o}", bufs=8)
                            sl = qt8[:, j * 4096 + c * 1024 + o * 512:
                                     j * 4096 + c * 1024 + (o + 1) * 512]
                            nc.vector.tensor_mul(sq[:], sl, sl)
                            lvl.append(sq)
                    depth = 0
                    while len(lvl) > 1:
                        nxt = []
                        for i in range(0, len(lvl), 2):
                            s = sqstream.tile([128, 512], BF16,
                                              tag=f"qsa{depth}",
                                              name=f"qsa{depth}_{i}",
                                              bufs=max(2, 4 >> depth))
                            nc.vector.tensor_add(s[:], lvl[i][:], lvl[i + 1][:])
                            nxt.append(s)
                        lvl = nxt
                        depth += 1
                    xdp = psum_xd.tile([1, 512], F32, tag="xdq")
                    nc.tensor.matmul(xdp[:], neghalf_col[:], lvl[0][:],
                                     start=True, stop=True)
                    nc.scalar.activation(xdT_q[0:1, c * 512:(c + 1) * 512],
                                         xdp[:], AF.Copy, bias=LOG64)
                for mt in range(MT):
                    jq, oq = divmod(mt, 2)
                    for c in range(NCH):
                        wqp = psum_wtx.tile([128, 512], F32, tag="wq")
                        for j in range(4):
                            nc.tensor.matmul(
                                wqp[:],
                                _pair(wq8[:, j * 1024 + mt * 256:
                                          j * 1024 + (mt + 1) * 256]),
                                _pair(qt8[:, j * 4096 + c * 1024:
                                          j * 4096 + (c + 1) * 1024]),
                                start=(j == 0), stop=False, perf_mode=DR)
                        nc.tensor.matmul(wqp[:], ones_row[:],
                                         xdT_q[0:1, c * 512:(c + 1) * 512],
                                         start=False, stop=True)
                        for rl in range(4):
                            r = c * 4 + rl
                            nc.scalar.activation(
                                qp8[:, jq * 4096 + r * 256 + oq * 128:
                                    jq * 4096 + r * 256 + (oq + 1) * 128],
                                wqp[:, rl * 128:(rl + 1) * 128], AF.Exp)

            # ---- HAM warm-keeper: paced dummy matmuls (CC insurance) ----
            with tc.tile_pool(name="psum_dummy", bufs=2,
                              space=bass.MemorySpace.PSUM) as psum_dummy:
                for i in range(N_DUMMY):
                    src, dst = (junkA, junkB) if i % 2 == 0 else (junkB, junkA)
                    nc.vector.tensor_copy(dst[:], src[:])
                    dp = psum_dummy.tile([128, 16], F32, tag="dp")
                    nc.tensor.matmul(dp[:], ident[:, 0:128],
                                     dst[:, 0:16], start=True, stop=True)

            # ---- OUT: out = po / (64*pD + 4096*m*eps) ----
            with (
                tc.tile_pool(name="psum_o", bufs=3,
                             space=bass.MemorySpace.PSUM) as psum_o,
                tc.tile_pool(name="psum_D", bufs=2,
                             space=bass.MemorySpace.PSUM) as psum_D,
            ):
                for r in range(RC):
                    po = psum_o.tile([128, D_MODEL], F32, tag="po")
                    pD = psum_D.tile([128, 1], F32, tag="pD")
                    for j in range(2):
                        lhs = _pair(qp8[:, j * 4096 + r * 256:
                                        j * 4096 + (r + 1) * 256])
                        for h in range(2):
                            nc.tensor.matmul(
                                po[:, h * 512:(h + 1) * 512], lhs,
                                _pair(C8[:, j * 2048 + h * 1024:
                                         j * 2048 + (h + 1) * 1024]),
                                start=(j == 0), stop=(j == 1), perf_mode=DR)
                    for mt in range(MT):
                        jq, oq = divmod(mt, 2)
                        nc.tensor.matmul(
                            pD[:],
                            qp8[:, jq * 4096 + r * 256 + oq * 128:
                                jq * 4096 + r * 256 + (oq + 1) * 128],
                            ksum8[:, mt:mt + 1],
                            start=(mt == 0), stop=(mt == MT - 1))
                    Dp = small.tile([128, 1], F32, tag="Dp")
                    recD = small.tile([128, 1], F32, tag="recD")
                    nc.scalar.activation(Dp[:], pD[:], AF.Copy,
                                         scale=DIV_SCALE, bias=DIV_BIAS)
                    nc.vector.reciprocal(recD[:], Dp[:])
                    ot = outp.tile([128, D_MODEL], BF16, tag="ot")
                    for h in range(2):
                        nc.vector.tensor_scalar_mul(
                            ot[:, h * 512:(h + 1) * 512],
                            po[:, h * 512:(h + 1) * 512], recD[:])
                    nc.sync.dma_start(out_d[r * 128:(r + 1) * 128, :], ot[:])

    nc.compile()
    return nc


_NC_CACHE = None


def _get_program():
    global _NC_CACHE
    if _NC_CACHE is None:
        _NC_CACHE = _build_program()
    return _NC_CACHE


def _c(a):
    return np.ascontiguousarray(a)


def _make_in_maps(q, k, v, w, proj_w):
    wT = w.T.astype(FP8_NP)          # [1024, 512]
    pwT = proj_w.T.astype(FP8_NP)    # [1024, 1024]
    wk = _c(wT.reshape(4, 2, 128, 512).transpose(2, 0, 1, 3)
            .reshape(128, 4096))
    wq = _c(wT.reshape(4, 2, 128, 4, 128).transpose(2, 0, 3, 1, 4)
            .reshape(128, 4096))
    pw = _c(pwT.reshape(4, 2, 128, 2, 512).transpose(2, 0, 3, 1, 4)
            .reshape(128, 8192))
    in_maps = []
    for c in range(N_CORES):
        b, h = divmod(c, 2)
        sl = slice(h * TC, (h + 1) * TC)
        kT = k[b, sl].T.astype(FP8_NP)   # [1024, 2048]
        qT = q[b, sl].T.astype(FP8_NP)
        vv = v[b, sl].astype(FP8_NP)     # [2048, 1024]
        in_maps.append({
            "k8": _c(kT.reshape(4, 2, 128, 16, 128).transpose(2, 3, 0, 1, 4)
                     .reshape(128, 16384)),
            "q8": _c(qT.reshape(4, 2, 128, 4, 512).transpose(2, 0, 3, 1, 4)
                     .reshape(128, 16384)),
            "v8": _c(vv.reshape(8, 2, 128, 8, 128).transpose(2, 0, 3, 1, 4)
                     .reshape(128, 16384)),
            "wk8": wk,
            "ident": np.eye(128, dtype=BF16_NP),
            "wq8": wq,
            "pw8": pw,
        })
    return in_maps


def run(q, k, v, w, proj_w, trace=False, tmpdir=None):
    nc = _get_program()
    in_maps = _make_in_maps(q, k, v, w, proj_w)
    res = bass_utils.run_bass_kernel_spmd(
        nc, in_maps, core_ids=list(range(N_CORES)), trace=trace,
        tmpdir=tmpdir)
    out = np.empty((B, T, D_MODEL), dtype=np.float32)
    for c in range(N_CORES):
        b, h = divmod(c, 2)
        out[b, h * TC:(h + 1) * TC] = res.results[c]["out"].astype(np.float32)
    return out, res


def kernel(q, k, v, w, proj_w):
    out, _ = run(np.asarray(q, dtype=np.float32),
                 np.asarray(k, dtype=np.float32),
                 np.asarray(v, dtype=np.float32),
                 np.asarray(w, dtype=np.float32),
                 np.asarray(proj_w, dtype=np.float32))
    return out
